# revision 18
# baseline (speedup 1.0000x reference)
"""BigBird attention kernel for 8 Trainium2 NeuronCores.

Sharding: data-parallel over batch (2) x tensor-parallel over heads (4 groups
of 4 heads) = 8 cores. Each core computes q/k/v projections for its head
slice, block-sparse masked attention, and a partial output projection with
its Wo row-slice; the host sums the 4 partial outputs per batch.

v6 design (vs the ~148us baseline):
- Phase-serial (QKV then attention) like the baseline: with scoped psum
  pools each phase gets the full 8-bank budget (qkv: 4 qk chains + rot +
  v; attention: 4 score banks + 2 shared av/Wo), which keeps the PE
  streaming - fused variants starved psum and lost more to per-group
  stalls than they saved in overlap.
- PE warmup: matmuls on a zero tile ramp the tensor-engine p-state and
  cover the input-DMA window; first real matmul starts ~8us in (vs 15).
- Host-packed [128, chunk, cols] inputs: one DMA per tensor (descriptor
  gen costs ~0.6us engine time per dma_start; the baseline burned ~30us
  of issue). x loads per 512-col band, band 0 split in 4 so the first
  chain starts as soon as its first chunks land.
- No identity-matmul mask preload (the baseline spent a full extra pass
  over the score columns on PE). The mask is applied as a post-exp 0/1
  multiply on DVE; exp inputs are ~N(0,1) so no -inf bias is needed.
- Column-uniform masked blocks (the global k<16 column for q-tiles >= 3)
  skip the DVE mask: their AV matmuls use a copy of v[kt] with the
  masked k-rows (and the ones column) zeroed.
- Scores contract 64 real head-dim partitions (no zero-padded k tiles).
- Scores are computed TRANSPOSED (S^T[k,q]); P^T feeds AV as the moving
  operand with V plus a ones column (softmax row-sum l for free)
  stationary; AV runs per-q-tile contiguous accumulation chains
  (interleaved chains within one psum bank corrupt results on TRN2).
  1/l via fast-approx DVE reciprocal (psum-path miscomputes), GpSimd
  partition_broadcast, folded into the psum->sbuf copy of O^T.
"""

import sys

for _p in ("/opt/trn_rl_repo", "/opt/trn_rl_repo/concourse"):
    if _p not in sys.path:
        sys.path.insert(0, _p)

import numpy as np

import concourse.bacc as bacc
import concourse.bass as bass
import concourse.mybir as mybir
import concourse.tile as tile
from concourse import bass_utils

F32 = mybir.dt.float32
BF16 = mybir.dt.bfloat16

B, S, D, H = 2, 2048, 1024, 16
HD = D // H          # 64
SCALE = 1.0 / float(np.sqrt(HD))
NCORES = 8
HG = 4               # head groups (tensor-parallel)
HPC = H // HG        # heads per core = 4
DC = HPC * HD        # channels per core = 256
QT = 128             # supertile edge
NQ = S // QT         # 16
NG = 4               # 512-col bands
KC = D // 128        # 8 contraction chunks
CC = DC // 128       # 2 channel chunks (2 heads each)


def _runs(bools):
    """Maximal [lo, hi) runs of True."""
    out = []
    lo = None
    for i, b in enumerate(list(bools) + [False]):
        if b and lo is None:
            lo = i
        elif not b and lo is not None:
            out.append((lo, i))
            lo = None
    return out


def _sched(mask):
    """Block-sparse schedule + mask/variant metadata from the runtime mask."""
    sup = mask.reshape(NQ, QT, NQ, QT).any(axis=(1, 3))  # [16,16]
    kts = [np.nonzero(sup[qi])[0].tolist() for qi in range(NQ)]
    kset = sorted({kt for qi in range(NQ) for kt in kts[qi]})
    ulo, uhi = {}, {}
    for kt in kset:
        us = [qi for qi in range(NQ) if kt in kts[qi]]
        ulo[kt], uhi[kt] = min(us), max(us)
    kts_eff = [[kt for kt in kset if ulo[kt] <= qi <= uhi[kt]]
               for qi in range(NQ)]
    assert all(kts_eff[qi] for qi in range(NQ)), "fully masked q row"

    # column-uniform blocks -> v-variant candidates
    pats = {}
    for qi in range(NQ):
        for kt in kts_eff[qi]:
            blk = mask[qi * QT:(qi + 1) * QT, kt * QT:(kt + 1) * QT]
            if blk.all():
                continue  # fully dense: nothing to mask anyway
            if np.all(blk == blk[0:1, :]):
                pat = blk[0]
                rr = _runs(pat)
                if 1 <= len(rr) <= 2:
                    pats.setdefault((kt, pat.tobytes()), (rr, set()))[1].add(qi)
    variants, var_of = [], {}
    for (kt, _pb), (rr, users) in sorted(pats.items(), key=lambda x: x[0][0]):
        if len(users) >= 2:
            vi = len(variants)
            variants.append((kt, rr, users))
            for qi in users:
                var_of[(qi, kt)] = vi

    chunks = []
    mask_blocks = []  # (qi, kt) in packed order
    mcols = 0
    maxgrp = 0
    for qig in range(NG):
        qlo0, qhi0 = 4 * qig, 4 * qig + 4
        bchunks = []
        boff = 0
        for kt in kset:
            qlo = max(ulo[kt], qlo0)
            qhi = min(uhi[kt], qhi0 - 1)
            if qlo > qhi:
                continue
            W = (qhi - qlo + 1) * QT
            bchunks.append((kt, qlo, qhi, W, boff))
            boff += W
        groups = []
        cur, curw = [], 0
        for ch in bchunks:
            if curw + ch[3] > 512:
                groups.append(cur)
                cur, curw = [], 0
            cur.append(ch)
            curw += ch[3]
        if cur:
            groups.append(cur)
        maxgrp = max(maxgrp, len(groups))

        # per-group additive-bias preload info + kt -> (group, offset,
        # qlo) map for AV.  A group needs a preload iff any of its
        # blocks is masked and not variant-served; preloads cover the
        # full group width (zeros on clean sub-blocks).
        moff0 = mcols
        bw = 0
        gpre = []  # per group: None | offset into band mask slice
        ktmap = {}
        for gi, grp in enumerate(groups):
            g0 = grp[0][4]
            gw = sum(c[3] for c in grp)
            need_pre = False
            for kt, qlo, qhi, W, bo in grp:
                ktmap[kt] = (gi, bo - g0, qlo)
                for qi in range(qlo, qhi + 1):
                    blk = mask[qi * QT:(qi + 1) * QT, kt * QT:(kt + 1) * QT]
                    if not blk.all() and var_of.get((qi, kt)) is None:
                        need_pre = True
            if need_pre:
                gpre.append(bw)
                for kt, qlo, qhi, W, bo in grp:
                    for qi in range(qlo, qhi + 1):
                        mask_blocks.append(
                            (qi, kt, var_of.get((qi, kt)) is not None))
                bw += gw
            else:
                gpre.append(None)
        mcols += bw
        chunks.append(dict(qlo=qlo0, qhi=qhi0, groups=groups,
                           gpre=gpre, mask_off=moff0, mask_w=bw,
                           ktmap=ktmap))

    return dict(kts_eff=kts_eff, chunks=chunks, mask_cols=mcols,
                mask_blocks=mask_blocks, variants=variants, var_of=var_of,
                maxgrp=maxgrp)


def _build_nc(sc):
    chunks = sc["chunks"]
    kts_eff = sc["kts_eff"]
    nvar = len(sc["variants"])

    nc = bacc.Bacc("TRN2", target_bir_lowering=False, debug=False)

    xc_d = nc.dram_tensor("xc", [128, KC, S], BF16, kind="ExternalInput")
    wq_d = nc.dram_tensor("wq", [128, KC, DC], BF16, kind="ExternalInput")
    wk_d = nc.dram_tensor("wk", [128, KC, DC], BF16, kind="ExternalInput")
    wv_d = nc.dram_tensor("wv", [128, KC, DC], BF16, kind="ExternalInput")
    wo_d = nc.dram_tensor("wo", [128, CC, D], BF16, kind="ExternalInput")
    cos_d = nc.dram_tensor("cosT", [128, S], BF16, kind="ExternalInput")
    sin_d = nc.dram_tensor("sinT", [128, S], BF16, kind="ExternalInput")
    rt_d = nc.dram_tensor("rT", [128, 128], BF16, kind="ExternalInput")
    id_d = nc.dram_tensor("ident", [128, 128], BF16, kind="ExternalInput")
    mcols = max(sc["mask_cols"], 128)
    mk_d = nc.dram_tensor("maskT", [128, mcols], BF16, kind="ExternalInput")
    out_d = nc.dram_tensor("out", [S, D], BF16, kind="ExternalOutput")

    with tile.TileContext(nc) as tc:
        from contextlib import ExitStack
        ctx = ExitStack()
        pp = ctx.enter_context(tc.tile_pool(name="persist", bufs=1))
        wp = ctx.enter_context(tc.tile_pool(name="weights", bufs=1))
        xp = ctx.enter_context(tc.tile_pool(name="xchunks", bufs=1))
        mp = ctx.enter_context(tc.tile_pool(name="maskp", bufs=1))
        sp = ctx.enter_context(tc.tile_pool(name="scratch", bufs=3))
        bp = ctx.enter_context(tc.tile_pool(name="probs", bufs=3))
        lr = ctx.enter_context(tc.tile_pool(name="lrec", bufs=2))
        otp = ctx.enter_context(tc.tile_pool(name="otile", bufs=2))
        obp = ctx.enter_context(tc.tile_pool(name="obuf", bufs=3))

        # ---- persistent tiles ----
        qbT = [pp.tile([128, S], BF16, tag=f"qbT{c}", name=f"qbT{c}")
               for c in range(CC)]
        kbZ = [pp.tile([128, S], BF16, tag=f"kbZ{h}", name=f"kbZ{h}")
               for h in range(HPC)]
        vb1 = [pp.tile([128, HPC, HD + 1], BF16, tag=f"v{i}", name=f"v{i}")
               for i in range(NQ)]
        vgs = [pp.tile([128, HPC, HD + 1], BF16, tag=f"vg{j}", name=f"vg{j}")
               for j in range(nvar)]

        wq_sb = wp.tile([128, KC, DC], BF16, tag="wq")
        wk_sb = wp.tile([128, KC, DC], BF16, tag="wk")
        wv_sb = wp.tile([128, KC, DC], BF16, tag="wv")
        wo_sb = wp.tile([128, CC, D], BF16, tag="wo")
        cosT = wp.tile([128, S], BF16, tag="cosT")
        sinT = wp.tile([128, S], BF16, tag="sinT")
        rT = wp.tile([128, 128], BF16, tag="rT")
        ident = wp.tile([128, 128], BF16, tag="ident")
        warm = wp.tile([128, 512], BF16, tag="warm")

        xb = [xp.tile([128, KC, 512], BF16, tag=f"xb{pc}",
                      name=f"xb{pc}") for pc in range(NG)]
        mbs = [mp.tile([128, max(ch["mask_w"], 128)], BF16, tag=f"mb{i}",
                       name=f"mb{i}") for i, ch in enumerate(chunks)]

        # ---- upfront DMA issue, spread across the 3 dma-capable queues ----
        nc.scalar.dma_start(wq_sb[:, 0:4, :], wq_d[:, 0:4, :])
        for j in range(4):
            nc.sync.dma_start(xb[0][:, 2 * j:2 * j + 2, :],
                              xc_d[:, 2 * j:2 * j + 2, 0:512])
        nc.scalar.dma_start(wk_sb[:, 0:4, :], wk_d[:, 0:4, :])
        nc.scalar.dma_start(wq_sb[:, 4:8, :], wq_d[:, 4:8, :])
        nc.scalar.dma_start(wk_sb[:, 4:8, :], wk_d[:, 4:8, :])
        nc.scalar.dma_start(wv_sb[:], wv_d[:, :, :])
        nc.gpsimd.dma_start(rT[:], rt_d[:, :])
        nc.gpsimd.dma_start(ident[:], id_d[:, :])
        nc.gpsimd.dma_start(cosT[:], cos_d[:, :])
        nc.gpsimd.dma_start(sinT[:], sin_d[:, :])
        nc.scalar.dma_start(xb[1][:], xc_d[:, :, 512:1024])
        nc.scalar.dma_start(xb[3][:], xc_d[:, :, 1536:2048])
        for i, ch in enumerate(chunks):
            if ch["mask_w"]:
                mo = ch["mask_off"]
                nc.sync.dma_start(mbs[i][:, :ch["mask_w"]],
                                  mk_d[:, mo:mo + ch["mask_w"]])
        nc.sync.dma_start(wo_sb[:], wo_d[:, :, :])
        nc.sync.dma_start(xb[2][:], xc_d[:, :, 1024:1536])
        for pi in range(NQ):
            nc.vector.memset(vb1[pi][:, :, HD:HD + 1], 1.0)
        nc.vector.memset(warm[:], 0.0)
        for h in range(HPC):
            zo = 64 - (h % 2) * 64  # the OTHER head's rows
            nc.gpsimd.memset(kbZ[h][zo:zo + 64, :], 0.0)

        # ---------------- QKV + RoPE (+v interleaved) ----------
        with (
            tc.tile_pool(name="qkv_ps", bufs=1, space="PSUM") as psp,
            tc.tile_pool(name="qkv_rot", bufs=2, space="PSUM") as psr,
            tc.tile_pool(name="qkv_psv", bufs=2, space="PSUM") as psv,
        ):
            # PE warmup: ramp the p-state / cover the DMA window
            for w in range(4):
                pswarm = psp.tile([128, 512], F32, tag="ps_qk0",
                                  name="pswarm")
                nc.tensor.matmul(pswarm[:], warm[:, 0:128], warm[:],
                                 start=True, stop=True)

            quads = [(cc, w_sb, tg) for cc in range(CC)
                     for w_sb, tg in ((wq_sb, "q"), (wk_sb, "k"))]
            for pc in range(NG):
                fs = slice(pc * 512, (pc + 1) * 512)
                pss4 = [psp.tile([128, 512], F32, tag=f"ps_qk{j}",
                                 name=f"ps_qk{j}") for j in range(4)]
                for k in range(KC):
                    for j, (ccq, w_sb, tg) in enumerate(quads):
                        nc.tensor.matmul(
                            pss4[j][:],
                            w_sb[:, k, ccq * 128:(ccq + 1) * 128],
                            xb[pc][:, k, :],
                            start=(k == 0), stop=(k == KC - 1))
                for j, (ccq, w_sb, tg) in enumerate(quads):
                    raw = sp.tile([128, 512], BF16, tag="raw", name="raw")
                    nc.scalar.copy(raw[:], pss4[j][:])
                    rot = psr.tile([128, 512], F32, tag="rot", name="rot")
                    nc.tensor.matmul(rot[:], rT[:], raw[:], start=True,
                                     stop=True)
                    u = sp.tile([128, 512], BF16, tag="u", name="u")
                    nc.vector.tensor_mul(u[:], rot[:], sinT[:, fs])
                    m = sp.tile([128, 512], BF16, tag="m", name="m")
                    nc.vector.tensor_mul(m[:], raw[:], cosT[:, fs])
                    if tg == "q":
                        nc.vector.tensor_add(qbT[ccq][:, fs], m[:], u[:])
                    else:
                        for h2 in range(2):
                            ho2 = h2 * 64
                            nc.vector.tensor_add(
                                kbZ[2 * ccq + h2][ho2:ho2 + 64, fs],
                                m[ho2:ho2 + 64, :], u[ho2:ho2 + 64, :])
                # v natural [128, 4, 65] per seq tile, interleaved per
                # band to fill the input-DMA window
                for pi in range(4 * pc, 4 * pc + 4):
                    ps_v = psv.tile([128, DC], F32, tag="ps_v",
                                    name="ps_v")
                    for k in range(KC):
                        nc.tensor.matmul(
                            ps_v[:],
                            xb[pc][:, k, (pi % 4) * 128:(pi % 4 + 1) * 128],
                            wv_sb[:, k, :],
                            start=(k == 0), stop=(k == KC - 1))
                    nc.vector.tensor_copy(vb1[pi][:, :, 0:HD], ps_v[:])
                if pc == 0:
                    for j, (kt, rowruns, _u) in enumerate(sc["variants"]):
                        nc.gpsimd.memset(vgs[j][:], 0.0)
                        for lo, hi in rowruns:
                            nc.gpsimd.tensor_copy(vgs[j][lo:hi, :, :],
                                                  vb1[kt][lo:hi, :, :])

        # ---------------- attention + output projection ---------
        with (
            tc.tile_pool(name="ps_sc", bufs=5, space="PSUM") as pss,
            tc.tile_pool(name="ps_apw", bufs=3, space="PSUM") as psa,
        ):
            for ci, ch in enumerate(chunks):
                qg0 = ch["qlo"]
                mb = mbs[ci]
                ot_sb = {cc: otp.tile([128, 512], BF16, tag=f"ot{cc}",
                                      name=f"ot{cc}") for cc in range(CC)}
                for hp in range(HPC // 2):
                    pbmaps = {}
                    for h in (2 * hp, 2 * hp + 1):
                        cc, ho = h // 2, (h % 2) * 64
                        pbg = []
                        for gi, grp in enumerate(ch["groups"]):
                            g0 = grp[0][4]
                            gw = sum(c[3] for c in grp)
                            pre = ch["gpre"][gi]
                            scp = pss.tile([128, 512], F32, tag="sc",
                                           name="scp")
                            for kt, qlo, qhi, W, bo in grp:
                                go = bo - g0
                                if pre is not None:
                                    # additive 0/-240 bias preloaded in
                                    # the same accumulation group
                                    nc.tensor.matmul(
                                        scp[:, go:go + W], ident[:],
                                        mb[:, pre + go:pre + go + W],
                                        start=True, stop=False)
                                nc.tensor.matmul(
                                    scp[:, go:go + W],
                                    kbZ[2 * cc + (h % 2)][
                                        :, kt * 128:(kt + 1) * 128],
                                    qbT[cc][:,
                                            qlo * 128:(qhi + 1) * 128],
                                    start=(pre is None), stop=True)
                            pb = bp.tile([128, 512], BF16, tag=f"pb{gi}",
                                         name=f"pb{gi}")
                            nc.scalar.activation(
                                pb[:, :gw], scp[:, :gw],
                                mybir.ActivationFunctionType.Exp,
                                bias=0.0, scale=SCALE)
                            pbg.append(pb)
                        pbmaps[h] = pbg
                    for h in (2 * hp, 2 * hp + 1):
                        cc, ho = h // 2, (h % 2) * 64
                        pbg = pbmaps[h]
                        av = psa.tile([128, 512], F32, tag="apw",
                                      name="av")
                        for qi in range(qg0, qg0 + 4):
                            co = (qi - qg0) * 128
                            for kt in kts_eff[qi]:
                                gi, go, qlo = ch["ktmap"][kt]
                                po = go + (qi - qlo) * 128
                                vi = sc["var_of"].get((qi, kt))
                                vb = vb1[kt] if vi is None else vgs[vi]
                                nc.tensor.matmul(
                                    av[0:65, co:co + 128],
                                    vb[:, h:h + 1, :],
                                    pbg[gi][:, po:po + 128],
                                    start=(kt == kts_eff[qi][0]),
                                    stop=(kt == kts_eff[qi][-1]))
                        lsb = lr.tile([1, 512], F32, tag="lsb", name="lsb")
                        nc.vector.tensor_copy(lsb[:], av[64:65, :])
                        rh = lr.tile([1, 512], F32, tag="rh", name="rh")
                        nc.vector.reciprocal_approx_fast(rh[:], lsb[:])
                        rb = lr.tile([64, 512], F32, tag="rb", name="rb")
                        nc.gpsimd.partition_broadcast(rb[:], rh[:])
                        nc.vector.tensor_mul(ot_sb[cc][ho:ho + 64, :],
                                             av[0:64, :], rb[:])
                for qi4 in range(4):
                    ob = obp.tile([128, D], BF16, tag="ob", name="ob")
                    for n2 in range(2):
                        pw = psa.tile([128, 512], F32, tag="apw",
                                      name="pw")
                        for cc2 in range(CC):
                            nc.tensor.matmul(
                                pw[:],
                                ot_sb[cc2][:, qi4 * 128:(qi4 + 1) * 128],
                                wo_sb[:, cc2, n2 * 512:(n2 + 1) * 512],
                                start=(cc2 == 0), stop=(cc2 == CC - 1))
                        if n2 == 0:
                            nc.scalar.copy(ob[:, 0:512], pw[:])
                        else:
                            nc.vector.tensor_copy(ob[:, 512:1024], pw[:])
                    qi = qg0 + qi4
                    nc.gpsimd.dma_start(out_d[qi * 128:(qi + 1) * 128, :],
                                        ob[:])

        ctx.close()

    nc.compile()
    return nc


def _host_inputs(x, freqs_cos, freqs_sin, position_ids, mask01, sc,
                 Wq, Wk, Wv, Wo):
    """Per-core input maps (chunk-packed layouts, see _build_nc)."""
    import ml_dtypes
    bf = ml_dtypes.bfloat16

    def chunkpack(w):  # [nch*128, N] -> [128, nch, N]
        nch = w.shape[0] // 128
        return np.ascontiguousarray(
            w.reshape(nch, 128, w.shape[1]).transpose(1, 0, 2)).astype(bf)

    r64 = np.zeros((HD, HD), np.float32)
    for i in range(HD // 2):
        r64[2 * i, 2 * i + 1] = -1.0
        r64[2 * i + 1, 2 * i] = 1.0
    r128 = np.zeros((128, 128), np.float32)
    r128[:64, :64] = r64
    r128[64:, 64:] = r64
    rT = np.ascontiguousarray(r128.T).astype(bf)

    # packed transposed additive mask bias: 0 where allowed or variant-
    # served, -240 where masked (exp -> ~0)
    mcols = max(sc["mask_cols"], 128)
    maskTc = np.zeros((128, mcols), bf)
    o = 0
    for qi, kt, isvar in sc["mask_blocks"]:
        if not isvar:
            blkT = (mask01[qi * QT:(qi + 1) * QT,
                           kt * QT:(kt + 1) * QT].T - 1.0) * 240.0
            maskTc[:, o:o + QT] = blkT
        o += QT
    assert o == sc["mask_cols"]

    in_maps = []
    for c in range(NCORES):
        b, g = c // HG, c % HG
        pos = np.clip(position_ids[b].astype(np.int64), 0,
                      freqs_cos.shape[0] - 1)
        cos_g = np.asarray(freqs_cos)[pos]  # [S, 32]
        sin_g = np.asarray(freqs_sin)[pos]
        cosT64 = np.repeat(cos_g.T, 2, axis=0)  # [64, S]
        sinT64 = np.repeat(sin_g.T, 2, axis=0)
        cs = slice(g * DC, (g + 1) * DC)
        in_maps.append({
            "xc": chunkpack(np.ascontiguousarray(x[b].T)),
            "wq": chunkpack(Wq[:, cs]),
            "wk": chunkpack(Wk[:, cs]),
            "wv": chunkpack(Wv[:, cs]),
            "wo": chunkpack(Wo[cs, :]),
            "cosT": np.concatenate([cosT64, cosT64], axis=0).astype(bf),
            "sinT": np.concatenate([sinT64, sinT64], axis=0).astype(bf),
            "rT": rT,
            "ident": np.eye(128, dtype=np.float32).astype(bf),
            "maskT": maskTc,
        })
    return in_maps


_CACHE = {}


def _get_nc(mask_key, sc):
    if mask_key not in _CACHE:
        _CACHE[mask_key] = _build_nc(sc)
    return _CACHE[mask_key]


def kernel(x, freqs_cos, freqs_sin, position_ids, bigbird_mask, Wq, Wk, Wv, Wo,
           _want_results=False, _trace=False, **trace_kwargs):
    x = np.asarray(x)
    mask = np.asarray(bigbird_mask).astype(bool)
    sc = _sched(mask)
    nc = _get_nc(mask.tobytes(), sc)
    in_maps = _host_inputs(
        x, np.asarray(freqs_cos), np.asarray(freqs_sin),
        np.asarray(position_ids), mask.astype(np.float32), sc,
        np.asarray(Wq), np.asarray(Wk), np.asarray(Wv), np.asarray(Wo),
    )
    res = bass_utils.run_bass_kernel_spmd(
        nc, in_maps, list(range(NCORES)), trace=_trace, **trace_kwargs
    )
    out = np.zeros((B, S, D), np.float32)
    for c in range(NCORES):
        out[c // HG] += res.results[c]["out"].astype(np.float32)
    if _want_results:
        return out, res
    return out


# revision 19
# speedup vs baseline: 1.1577x; 1.1577x over previous
"""BigBird attention kernel for 8 Trainium2 NeuronCores.

Sharding: data-parallel over batch (2) x tensor-parallel over heads (4 groups
of 4 heads) = 8 cores. Each core computes q/k/v projections for its head
slice, block-sparse masked attention, and a partial output projection with
its Wo row-slice; the host sums the 4 partial outputs per batch.

v6 design (vs the ~148us baseline):
- Phase-serial (QKV then attention) like the baseline: with scoped psum
  pools each phase gets the full 8-bank budget (qkv: 4 qk chains + rot +
  v; attention: 4 score banks + 2 shared av/Wo), which keeps the PE
  streaming - fused variants starved psum and lost more to per-group
  stalls than they saved in overlap.
- PE warmup: matmuls on a zero tile ramp the tensor-engine p-state and
  cover the input-DMA window; first real matmul starts ~8us in (vs 15).
- Host-packed [128, chunk, cols] inputs: one DMA per tensor (descriptor
  gen costs ~0.6us engine time per dma_start; the baseline burned ~30us
  of issue). x loads per 512-col band, band 0 split in 4 so the first
  chain starts as soon as its first chunks land.
- No identity-matmul mask preload (the baseline spent a full extra pass
  over the score columns on PE). The mask is applied as a post-exp 0/1
  multiply on DVE; exp inputs are ~N(0,1) so no -inf bias is needed.
- Column-uniform masked blocks (the global k<16 column for q-tiles >= 3)
  skip the DVE mask: their AV matmuls use a copy of v[kt] with the
  masked k-rows (and the ones column) zeroed.
- Scores contract 64 real head-dim partitions (no zero-padded k tiles).
- Scores are computed TRANSPOSED (S^T[k,q]); P^T feeds AV as the moving
  operand with V plus a ones column (softmax row-sum l for free)
  stationary; AV runs per-q-tile contiguous accumulation chains
  (interleaved chains within one psum bank corrupt results on TRN2).
  1/l via fast-approx DVE reciprocal (psum-path miscomputes), GpSimd
  partition_broadcast, folded into the psum->sbuf copy of O^T.
"""

import sys

for _p in ("/opt/trn_rl_repo", "/opt/trn_rl_repo/concourse"):
    if _p not in sys.path:
        sys.path.insert(0, _p)

import numpy as np

import concourse.bacc as bacc
import concourse.bass as bass
import concourse.mybir as mybir
import concourse.tile as tile
from concourse import bass_utils

F32 = mybir.dt.float32
BF16 = mybir.dt.bfloat16

B, S, D, H = 2, 2048, 1024, 16
HD = D // H          # 64
SCALE = 1.0 / float(np.sqrt(HD))
NCORES = 8
HG = 4               # head groups (tensor-parallel)
HPC = H // HG        # heads per core = 4
DC = HPC * HD        # channels per core = 256
QT = 128             # supertile edge
NQ = S // QT         # 16
NG = 4               # 512-col bands
KC = D // 128        # 8 contraction chunks
CC = DC // 128       # 2 channel chunks (2 heads each)


def _runs(bools):
    """Maximal [lo, hi) runs of True."""
    out = []
    lo = None
    for i, b in enumerate(list(bools) + [False]):
        if b and lo is None:
            lo = i
        elif not b and lo is not None:
            out.append((lo, i))
            lo = None
    return out


def _sched(mask):
    """Block-sparse schedule + mask/variant metadata from the runtime mask."""
    sup = mask.reshape(NQ, QT, NQ, QT).any(axis=(1, 3))  # [16,16]
    kts = [np.nonzero(sup[qi])[0].tolist() for qi in range(NQ)]
    kset = sorted({kt for qi in range(NQ) for kt in kts[qi]})
    ulo, uhi = {}, {}
    for kt in kset:
        us = [qi for qi in range(NQ) if kt in kts[qi]]
        ulo[kt], uhi[kt] = min(us), max(us)
    kts_eff = [[kt for kt in kset if ulo[kt] <= qi <= uhi[kt]]
               for qi in range(NQ)]
    assert all(kts_eff[qi] for qi in range(NQ)), "fully masked q row"

    # column-uniform blocks -> v-variant candidates
    pats = {}
    for qi in range(NQ):
        for kt in kts_eff[qi]:
            blk = mask[qi * QT:(qi + 1) * QT, kt * QT:(kt + 1) * QT]
            if blk.all():
                continue  # fully dense: nothing to mask anyway
            if np.all(blk == blk[0:1, :]):
                pat = blk[0]
                rr = _runs(pat)
                if 1 <= len(rr) <= 2:
                    pats.setdefault((kt, pat.tobytes()), (rr, set()))[1].add(qi)
    variants, var_of = [], {}
    for (kt, _pb), (rr, users) in sorted(pats.items(), key=lambda x: x[0][0]):
        if len(users) >= 2:
            vi = len(variants)
            variants.append((kt, rr, users))
            for qi in users:
                var_of[(qi, kt)] = vi

    chunks = []
    mask_blocks = []  # (qi, kt) in packed order
    mcols = 0
    maxgrp = 0
    for qig in range(NG):
        qlo0, qhi0 = 4 * qig, 4 * qig + 4
        bchunks = []
        boff = 0
        for kt in kset:
            qlo = max(ulo[kt], qlo0)
            qhi = min(uhi[kt], qhi0 - 1)
            if qlo > qhi:
                continue
            W = (qhi - qlo + 1) * QT
            bchunks.append((kt, qlo, qhi, W, boff))
            boff += W
        groups = []
        cur, curw = [], 0
        for ch in bchunks:
            if curw + ch[3] > 512:
                groups.append(cur)
                cur, curw = [], 0
            cur.append(ch)
            curw += ch[3]
        if cur:
            groups.append(cur)
        maxgrp = max(maxgrp, len(groups))

        # per-group additive-bias preload info + kt -> (group, offset,
        # qlo) map for AV.  A group needs a preload iff any of its
        # blocks is masked and not variant-served; preloads cover the
        # full group width (zeros on clean sub-blocks).
        moff0 = mcols
        bw = 0
        gpre = []  # per group: None | offset into band mask slice
        ktmap = {}
        for gi, grp in enumerate(groups):
            g0 = grp[0][4]
            gw = sum(c[3] for c in grp)
            need_pre = False
            for kt, qlo, qhi, W, bo in grp:
                ktmap[kt] = (gi, bo - g0, qlo)
                for qi in range(qlo, qhi + 1):
                    blk = mask[qi * QT:(qi + 1) * QT, kt * QT:(kt + 1) * QT]
                    if not blk.all() and var_of.get((qi, kt)) is None:
                        need_pre = True
            if need_pre:
                gpre.append(bw)
                for kt, qlo, qhi, W, bo in grp:
                    for qi in range(qlo, qhi + 1):
                        mask_blocks.append(
                            (qi, kt, var_of.get((qi, kt)) is not None))
                bw += gw
            else:
                gpre.append(None)
        mcols += bw
        chunks.append(dict(qlo=qlo0, qhi=qhi0, groups=groups,
                           gpre=gpre, mask_off=moff0, mask_w=bw,
                           ktmap=ktmap))

    return dict(kts_eff=kts_eff, chunks=chunks, mask_cols=mcols,
                mask_blocks=mask_blocks, variants=variants, var_of=var_of,
                maxgrp=maxgrp)


def _build_nc(sc):
    chunks = sc["chunks"]
    kts_eff = sc["kts_eff"]
    nvar = len(sc["variants"])

    nc = bacc.Bacc("TRN2", target_bir_lowering=False, debug=False)

    xc_d = nc.dram_tensor("xc", [128, KC, S], BF16, kind="ExternalInput")
    wq_d = nc.dram_tensor("wq", [128, KC, DC], BF16, kind="ExternalInput")
    wk_d = nc.dram_tensor("wk", [128, KC, DC], BF16, kind="ExternalInput")
    wv_d = nc.dram_tensor("wv", [128, KC, DC], BF16, kind="ExternalInput")
    wo_d = nc.dram_tensor("wo", [128, CC, D], BF16, kind="ExternalInput")
    cos_d = nc.dram_tensor("cosT", [128, S], BF16, kind="ExternalInput")
    sin_d = nc.dram_tensor("sinT", [128, S], BF16, kind="ExternalInput")
    rt_d = nc.dram_tensor("rT", [128, 128], BF16, kind="ExternalInput")
    id_d = nc.dram_tensor("ident", [128, 128], BF16, kind="ExternalInput")
    mcols = max(sc["mask_cols"], 128)
    mk_d = nc.dram_tensor("maskT", [128, mcols], BF16, kind="ExternalInput")
    out_d = nc.dram_tensor("out", [S, D], BF16, kind="ExternalOutput")

    with tile.TileContext(nc) as tc:
        from contextlib import ExitStack
        ctx = ExitStack()
        pp = ctx.enter_context(tc.tile_pool(name="persist", bufs=1))
        wp = ctx.enter_context(tc.tile_pool(name="weights", bufs=1))
        xp = ctx.enter_context(tc.tile_pool(name="xchunks", bufs=1))
        mp = ctx.enter_context(tc.tile_pool(name="maskp", bufs=1))
        sp = ctx.enter_context(tc.tile_pool(name="scratch", bufs=3))
        bp = ctx.enter_context(tc.tile_pool(name="probs", bufs=3))
        lr = ctx.enter_context(tc.tile_pool(name="lrec", bufs=2))
        otp = ctx.enter_context(tc.tile_pool(name="otile", bufs=2))
        obp = ctx.enter_context(tc.tile_pool(name="obuf", bufs=3))

        # ---- persistent tiles ----
        qbT = [pp.tile([128, S], BF16, tag=f"qbT{c}", name=f"qbT{c}")
               for c in range(CC)]
        kbZ = [pp.tile([128, S], BF16, tag=f"kbZ{h}", name=f"kbZ{h}")
               for h in range(HPC)]
        vb1 = [pp.tile([128, HPC, HD + 1], BF16, tag=f"v{i}", name=f"v{i}")
               for i in range(NQ)]
        vgs = [pp.tile([128, HPC, HD + 1], BF16, tag=f"vg{j}", name=f"vg{j}")
               for j in range(nvar)]

        wq_sb = wp.tile([128, KC, DC], BF16, tag="wq")
        wk_sb = wp.tile([128, KC, DC], BF16, tag="wk")
        wv_sb = wp.tile([128, KC, DC], BF16, tag="wv")
        wo_sb = wp.tile([128, CC, D], BF16, tag="wo")
        cosT = wp.tile([128, S], BF16, tag="cosT")
        sinT = wp.tile([128, S], BF16, tag="sinT")
        rT = wp.tile([128, 128], BF16, tag="rT")
        ident = wp.tile([128, 128], BF16, tag="ident")
        warm = wp.tile([128, 512], BF16, tag="warm")

        xb = [xp.tile([128, KC, 512], BF16, tag=f"xb{pc}",
                      name=f"xb{pc}") for pc in range(NG)]
        mbs = [mp.tile([128, max(ch["mask_w"], 128)], BF16, tag=f"mb{i}",
                       name=f"mb{i}") for i, ch in enumerate(chunks)]

        # ---- upfront DMA issue, spread across the 3 dma-capable queues ----
        nc.scalar.dma_start(wq_sb[:, 0:4, :], wq_d[:, 0:4, :])
        for j in range(4):
            nc.sync.dma_start(xb[0][:, 2 * j:2 * j + 2, :],
                              xc_d[:, 2 * j:2 * j + 2, 0:512])
        nc.scalar.dma_start(wk_sb[:, 0:4, :], wk_d[:, 0:4, :])
        nc.scalar.dma_start(wq_sb[:, 4:8, :], wq_d[:, 4:8, :])
        nc.scalar.dma_start(wk_sb[:, 4:8, :], wk_d[:, 4:8, :])
        nc.scalar.dma_start(wv_sb[:], wv_d[:, :, :])
        nc.gpsimd.dma_start(rT[:], rt_d[:, :])
        nc.gpsimd.dma_start(ident[:], id_d[:, :])
        nc.gpsimd.dma_start(cosT[:], cos_d[:, :])
        nc.gpsimd.dma_start(sinT[:], sin_d[:, :])
        nc.scalar.dma_start(xb[1][:], xc_d[:, :, 512:1024])
        nc.scalar.dma_start(xb[3][:], xc_d[:, :, 1536:2048])
        for i, ch in enumerate(chunks):
            if ch["mask_w"]:
                mo = ch["mask_off"]
                nc.sync.dma_start(mbs[i][:, :ch["mask_w"]],
                                  mk_d[:, mo:mo + ch["mask_w"]])
        nc.sync.dma_start(wo_sb[:], wo_d[:, :, :])
        nc.sync.dma_start(xb[2][:], xc_d[:, :, 1024:1536])
        for pi in range(NQ):
            nc.vector.memset(vb1[pi][:, :, HD:HD + 1], 1.0)
        nc.vector.memset(warm[:], 0.0)
        for h in range(HPC):
            zo = 64 - (h % 2) * 64  # the OTHER head's rows
            nc.gpsimd.memset(kbZ[h][zo:zo + 64, :], 0.0)

        # ---------------- QKV + RoPE (+v interleaved) ----------
        with (
            tc.tile_pool(name="qkv_ps", bufs=1, space="PSUM") as psp,
            tc.tile_pool(name="qkv_rot", bufs=2, space="PSUM") as psr,
            tc.tile_pool(name="qkv_psv", bufs=2, space="PSUM") as psv,
        ):
            # PE warmup: ramp the p-state / cover the DMA window
            for w in range(4):
                pswarm = psp.tile([128, 512], F32, tag="ps_qk0",
                                  name="pswarm")
                nc.tensor.matmul(pswarm[:], warm[:, 0:128], warm[:],
                                 start=True, stop=True)

            quads = [(cc, w_sb, tg) for cc in range(CC)
                     for w_sb, tg in ((wq_sb, "q"), (wk_sb, "k"))]
            for pc in range(NG):
                fs = slice(pc * 512, (pc + 1) * 512)
                pss4 = [psp.tile([128, 512], F32, tag=f"ps_qk{j}",
                                 name=f"ps_qk{j}") for j in range(4)]
                for k in range(KC):
                    for j, (ccq, w_sb, tg) in enumerate(quads):
                        nc.tensor.matmul(
                            pss4[j][:],
                            w_sb[:, k, ccq * 128:(ccq + 1) * 128],
                            xb[pc][:, k, :],
                            start=(k == 0), stop=(k == KC - 1))
                for j, (ccq, w_sb, tg) in enumerate(quads):
                    raw = sp.tile([128, 512], BF16, tag="raw", name="raw")
                    nc.scalar.copy(raw[:], pss4[j][:])
                    rot = psr.tile([128, 512], F32, tag="rot", name="rot")
                    nc.tensor.matmul(rot[:], rT[:], raw[:], start=True,
                                     stop=True)
                    u = sp.tile([128, 512], BF16, tag="u", name="u")
                    nc.vector.tensor_mul(u[:], rot[:], sinT[:, fs])
                    m = sp.tile([128, 512], BF16, tag="m", name="m")
                    nc.vector.tensor_mul(m[:], raw[:], cosT[:, fs])
                    if tg == "q":
                        nc.vector.tensor_add(qbT[ccq][:, fs], m[:], u[:])
                    else:
                        for h2 in range(2):
                            ho2 = h2 * 64
                            nc.vector.tensor_add(
                                kbZ[2 * ccq + h2][ho2:ho2 + 64, fs],
                                m[ho2:ho2 + 64, :], u[ho2:ho2 + 64, :])
                # v natural [128, 4, 65] per seq tile, interleaved per
                # band to fill the input-DMA window
                for pi in range(4 * pc, 4 * pc + 4):
                    ps_v = psv.tile([128, DC], F32, tag="ps_v",
                                    name="ps_v")
                    for k in range(KC):
                        nc.tensor.matmul(
                            ps_v[:],
                            xb[pc][:, k, (pi % 4) * 128:(pi % 4 + 1) * 128],
                            wv_sb[:, k, :],
                            start=(k == 0), stop=(k == KC - 1))
                    nc.vector.tensor_copy(vb1[pi][:, :, 0:HD], ps_v[:])
                if pc == 0:
                    for j, (kt, rowruns, _u) in enumerate(sc["variants"]):
                        nc.gpsimd.memset(vgs[j][:], 0.0)
                        for lo, hi in rowruns:
                            nc.gpsimd.tensor_copy(vgs[j][lo:hi, :, :],
                                                  vb1[kt][lo:hi, :, :])

        # ---------------- attention + output projection ---------
        with (
            tc.tile_pool(name="ps_sc", bufs=4, space="PSUM") as pss,
            tc.tile_pool(name="ps_apw", bufs=2, space="PSUM") as psa,
        ):
            for ci, ch in enumerate(chunks):
                qg0 = ch["qlo"]
                mb = mbs[ci]
                ot_sb = {cc: otp.tile([128, 512], BF16, tag=f"ot{cc}",
                                      name=f"ot{cc}") for cc in range(CC)}
                for hp in range(HPC // 2):
                    pbmaps = {}
                    for h in (2 * hp, 2 * hp + 1):
                        cc, ho = h // 2, (h % 2) * 64
                        pbg = []
                        for gi, grp in enumerate(ch["groups"]):
                            g0 = grp[0][4]
                            gw = sum(c[3] for c in grp)
                            pre = ch["gpre"][gi]
                            scp = pss.tile([128, 512], F32, tag="sc",
                                           name="scp")
                            for kt, qlo, qhi, W, bo in grp:
                                go = bo - g0
                                if pre is not None:
                                    # additive 0/-240 bias preloaded in
                                    # the same accumulation group
                                    nc.tensor.matmul(
                                        scp[:, go:go + W], ident[:],
                                        mb[:, pre + go:pre + go + W],
                                        start=True, stop=False)
                                nc.tensor.matmul(
                                    scp[:, go:go + W],
                                    kbZ[2 * cc + (h % 2)][
                                        :, kt * 128:(kt + 1) * 128],
                                    qbT[cc][:,
                                            qlo * 128:(qhi + 1) * 128],
                                    start=(pre is None), stop=True)
                            pb = bp.tile([128, 512], BF16, tag=f"pb{gi}",
                                         name=f"pb{gi}")
                            nc.scalar.activation(
                                pb[:, :gw], scp[:, :gw],
                                mybir.ActivationFunctionType.Exp,
                                bias=0.0, scale=SCALE)
                            pbg.append(pb)
                        pbmaps[h] = pbg
                    for h in (2 * hp, 2 * hp + 1):
                        cc, ho = h // 2, (h % 2) * 64
                        pbg = pbmaps[h]
                        av = psa.tile([128, 512], F32, tag="apw",
                                      name="av")
                        for qi in range(qg0, qg0 + 4):
                            co = (qi - qg0) * 128
                            for kt in kts_eff[qi]:
                                gi, go, qlo = ch["ktmap"][kt]
                                po = go + (qi - qlo) * 128
                                vi = sc["var_of"].get((qi, kt))
                                vb = vb1[kt] if vi is None else vgs[vi]
                                nc.tensor.matmul(
                                    av[0:65, co:co + 128],
                                    vb[:, h:h + 1, :],
                                    pbg[gi][:, po:po + 128],
                                    start=(kt == kts_eff[qi][0]),
                                    stop=(kt == kts_eff[qi][-1]))
                        lsb = lr.tile([1, 512], F32, tag="lsb", name="lsb")
                        nc.vector.tensor_copy(lsb[:], av[64:65, :])
                        rh = lr.tile([1, 512], F32, tag="rh", name="rh")
                        nc.vector.reciprocal_approx_fast(rh[:], lsb[:])
                        rb = lr.tile([64, 512], F32, tag="rb", name="rb")
                        nc.gpsimd.partition_broadcast(rb[:], rh[:])
                        nc.vector.tensor_mul(ot_sb[cc][ho:ho + 64, :],
                                             av[0:64, :], rb[:])
                for qi4 in range(4):
                    ob = obp.tile([128, D], BF16, tag="ob", name="ob")
                    for n2 in range(2):
                        pw = psa.tile([128, 512], F32, tag="apw",
                                      name="pw")
                        for cc2 in range(CC):
                            nc.tensor.matmul(
                                pw[:],
                                ot_sb[cc2][:, qi4 * 128:(qi4 + 1) * 128],
                                wo_sb[:, cc2, n2 * 512:(n2 + 1) * 512],
                                start=(cc2 == 0), stop=(cc2 == CC - 1))
                        if n2 == 0:
                            nc.scalar.copy(ob[:, 0:512], pw[:])
                        else:
                            nc.vector.tensor_copy(ob[:, 512:1024], pw[:])
                    qi = qg0 + qi4
                    nc.gpsimd.dma_start(out_d[qi * 128:(qi + 1) * 128, :],
                                        ob[:])

        ctx.close()

    nc.compile()
    return nc


def _host_inputs(x, freqs_cos, freqs_sin, position_ids, mask01, sc,
                 Wq, Wk, Wv, Wo):
    """Per-core input maps (chunk-packed layouts, see _build_nc)."""
    import ml_dtypes
    bf = ml_dtypes.bfloat16

    def chunkpack(w):  # [nch*128, N] -> [128, nch, N]
        nch = w.shape[0] // 128
        return np.ascontiguousarray(
            w.reshape(nch, 128, w.shape[1]).transpose(1, 0, 2)).astype(bf)

    r64 = np.zeros((HD, HD), np.float32)
    for i in range(HD // 2):
        r64[2 * i, 2 * i + 1] = -1.0
        r64[2 * i + 1, 2 * i] = 1.0
    r128 = np.zeros((128, 128), np.float32)
    r128[:64, :64] = r64
    r128[64:, 64:] = r64
    rT = np.ascontiguousarray(r128.T).astype(bf)

    # packed transposed additive mask bias: 0 where allowed or variant-
    # served, -240 where masked (exp -> ~0)
    mcols = max(sc["mask_cols"], 128)
    maskTc = np.zeros((128, mcols), bf)
    o = 0
    for qi, kt, isvar in sc["mask_blocks"]:
        if not isvar:
            blkT = (mask01[qi * QT:(qi + 1) * QT,
                           kt * QT:(kt + 1) * QT].T - 1.0) * 240.0
            maskTc[:, o:o + QT] = blkT
        o += QT
    assert o == sc["mask_cols"]

    in_maps = []
    for c in range(NCORES):
        b, g = c // HG, c % HG
        pos = np.clip(position_ids[b].astype(np.int64), 0,
                      freqs_cos.shape[0] - 1)
        cos_g = np.asarray(freqs_cos)[pos]  # [S, 32]
        sin_g = np.asarray(freqs_sin)[pos]
        cosT64 = np.repeat(cos_g.T, 2, axis=0)  # [64, S]
        sinT64 = np.repeat(sin_g.T, 2, axis=0)
        cs = slice(g * DC, (g + 1) * DC)
        in_maps.append({
            "xc": chunkpack(np.ascontiguousarray(x[b].T)),
            "wq": chunkpack(Wq[:, cs]),
            "wk": chunkpack(Wk[:, cs]),
            "wv": chunkpack(Wv[:, cs]),
            "wo": chunkpack(Wo[cs, :]),
            "cosT": np.concatenate([cosT64, cosT64], axis=0).astype(bf),
            "sinT": np.concatenate([sinT64, sinT64], axis=0).astype(bf),
            "rT": rT,
            "ident": np.eye(128, dtype=np.float32).astype(bf),
            "maskT": maskTc,
        })
    return in_maps


_CACHE = {}


def _get_nc(mask_key, sc):
    if mask_key not in _CACHE:
        _CACHE[mask_key] = _build_nc(sc)
    return _CACHE[mask_key]


def kernel(x, freqs_cos, freqs_sin, position_ids, bigbird_mask, Wq, Wk, Wv, Wo,
           _want_results=False, _trace=False, **trace_kwargs):
    x = np.asarray(x)
    mask = np.asarray(bigbird_mask).astype(bool)
    sc = _sched(mask)
    nc = _get_nc(mask.tobytes(), sc)
    in_maps = _host_inputs(
        x, np.asarray(freqs_cos), np.asarray(freqs_sin),
        np.asarray(position_ids), mask.astype(np.float32), sc,
        np.asarray(Wq), np.asarray(Wk), np.asarray(Wv), np.asarray(Wo),
    )
    res = bass_utils.run_bass_kernel_spmd(
        nc, in_maps, list(range(NCORES)), trace=_trace, **trace_kwargs
    )
    out = np.zeros((B, S, D), np.float32)
    for c in range(NCORES):
        out[c // HG] += res.results[c]["out"].astype(np.float32)
    if _want_results:
        return out, res
    return out
